# revision 2
# baseline (speedup 1.0000x reference)
"""Trainium2 Bass kernel for nn_DenseAttentionOneHead — pipelined variant.

out_b = X_b (W^T (X_b^T X_b)).  Column-sliced across 4 cores per batch
(256 output columns each), collective-free.  Per core:
  S_sl = X_b^T X_b[:, sl]          ([1024, 256])
  M_sl = W^T S_sl                  ([1024, 256])
  outT_sl = M_sl^T X_b^T           ([256, 4096], written fp16; host
                                    transposes back)

The kernel is HBM/PE co-limited (~20MB DMA, ~61us of matmul columns), so
everything hangs on keeping both engines continuously busy:
  - The X row stream owns the full HBM bandwidth during the S phase; the
    W and X^T loads are explicitly dependency-gated (add_dep_helper) on
    late S-phase matmuls so they drain in strict consumption order
    behind the stream instead of competing with it.
  - X^T arrives in quarter-major order and the out phase consumes it
    quarter by quarter, so out-phase compute starts ~6us after M.
  - The out phase computes outT with M_sl halves stationary and X^T as
    the 512-wide moving operand; output is fp16 (2MB not 4MB).
  - PE never idles >1us, so the HAM clock gate stays at 8/8.
"""

import numpy as np

import concourse.mybir as mybir
import concourse.tile as tile
from concourse import bacc
from concourse.bass import _add_dep_helper
from concourse.bass_utils import run_bass_kernel_spmd

F32 = mybir.dt.float32
F16 = mybir.dt.float16
P = 128
D = 1024
B = 2
N = 4096
NCORES = 8
GROUP = 4            # cores per batch
SL = D // GROUP      # 256-column slice per core
NO = D // P          # 8 blocks of 128 along D
NCH = N // P         # 32 row chunks of the batch
NQ = 4               # X^T quarters (1024 N-columns each)
QW = N // NQ         # 1024
ACT_COPY = mybir.ActivationFunctionType.Copy

_compiled = None


def _build():
    nc = bacc.Bacc(None, target_bir_lowering=False, debug=False, num_devices=NCORES)

    # xf arrives column-rotated per core (its 256 target columns first) and
    # wf row-rotated identically, so the same program computes every slice.
    xf = nc.dram_tensor("xf", [N, D], F16, kind="ExternalInput")
    xt = nc.dram_tensor("xt", [D, N], F16, kind="ExternalInput")
    wf = nc.dram_tensor("wf", [D, D], F16, kind="ExternalInput")
    o_out = nc.dram_tensor("o_out", [SL, N], F16, kind="ExternalOutput")

    xfv = xf.rearrange("(c p) d -> p c d", p=P)      # [128, 32, 1024]
    xtv = xt.rearrange("(c p) n -> p c n", p=P)      # [128, 8, 4096]
    wfv = wf.rearrange("(c p) a -> p c a", p=P)      # [128, 8, 1024]

    # X row-stream pieces: 1-chunk pieces through the DMA ramp-up, then
    # 2-chunk (512KB) pieces.  Uniform small pieces keep the two rings'
    # delivery aligned with consumption order (a large piece at the head
    # of one ring stalls the in-order consumer while the other ring
    # races ahead).
    piece_chunks = [1] * 4 + [2] * 14
    assert sum(piece_chunks) == NCH

    with tile.TileContext(nc) as tc:
        with (
            tc.tile_pool(name="big", bufs=1) as big,
            tc.tile_pool(name="xin", bufs=12) as xin,
            tc.tile_pool(name="stage", bufs=4) as stage,
            tc.tile_pool(name="psum", bufs=8, space="PSUM") as psum,
        ):
            Wsb = big.tile([P, NO, D], F16, tag="W")        # W [e, a], 2MB
            Ssl = big.tile([P, NO, SL], F16, tag="Ssl")     # S_sl [e, d_sl]
            Msl = big.tile([P, NO, SL], F16, tag="Msl")     # M_sl [a, d_sl]
            Xtq = [
                big.tile([P, NO, QW], F16, tag=f"Xtq{q}", name=f"Xtq{q}")
                for q in range(NQ)
            ]                                               # X^T quarters, 4x2MB

            # PE warm-up: ~3.5us of throwaway matmuls with no DMA dependency
            # run during the framework preamble / first-piece latency, so the
            # HAM clock gate reaches 8/8 before the first real matmul.
            dummy = big.tile([P, P], F16, tag="dummy")
            nc.gpsimd.memset(dummy[:], 0.0)
            dacc = psum.tile([P, 512], F32, tag="acc", name="warmup")
            for i in range(16):
                nc.tensor.matmul(
                    dacc[:, :P], dummy[:], dummy[:], start=(i == 0), stop=(i == 15)
                )

            # ---- S_sl = X^T X[:, sl]: stream X row-chunks on both rings
            accs = [
                psum.tile([P, 512], F32, tag="acc", name=f"sacc_{et}")[:, :SL]
                for et in range(NO)
            ]
            chunk_mm = {}       # chunk index -> its last matmul instruction
            ch = 0
            for pi, nch in enumerate(piece_chunks):
                xp = xin.tile([P, 2, D], F16, tag="xp")
                if pi == 0:
                    # Split the first chunk into column halves so its first
                    # matmuls start ~1.7us earlier (smaller first transfer).
                    nc.sync.dma_start(xp[:, 0, : D // 2], xfv[:, 0, : D // 2])
                    nc.scalar.dma_start(xp[:, 0, D // 2 :], xfv[:, 0, D // 2 :])
                else:
                    eng = nc.sync if pi % 2 == 0 else nc.scalar
                    eng.dma_start(xp[:, :nch, :], xfv[:, ch : ch + nch, :])
                for c in range(nch):
                    for et in range(NO):
                        mm = nc.tensor.matmul(
                            accs[et][:],
                            xp[:, c, et * P : (et + 1) * P],
                            xp[:, c, :SL],
                            start=(ch == 0),
                            stop=(ch == NCH - 1),
                        )
                    chunk_mm[ch] = mm
                    ch += 1

            # W and X^T loads, gated behind the stream in consumption order.
            gate = lambda dma, mm: _add_dep_helper(
                dma.ins, mm.ins, sync=True, reason="serialize HBM behind X stream"
            )
            # W and X^T loads trail the stream in strict consumption order
            # (each quarter split across both rings so quarters complete
            # sequentially).  q0/q1 go eagerly right behind W; q2/q3 are
            # gated off M-phase progress, moving ~4MB out of the window
            # where both HBM-pair cores burst simultaneously.
            d = nc.sync.dma_start(Wsb[:, :, : D // 2], wfv[:, :, : D // 2])
            gate(d, chunk_mm[16])
            d = nc.scalar.dma_start(Wsb[:, :, D // 2 :], wfv[:, :, D // 2 :])
            gate(d, chunk_mm[18])
            xtq_dma = []
            for q in range(NQ):
                da = nc.sync.dma_start(
                    Xtq[q][:, : NO // 2, :],
                    xtv[:, : NO // 2, q * QW : (q + 1) * QW],
                )
                db = nc.scalar.dma_start(
                    Xtq[q][:, NO // 2 :, :],
                    xtv[:, NO // 2 :, q * QW : (q + 1) * QW],
                )
                xtq_dma.append((da, db))
            gate(xtq_dma[0][0], chunk_mm[20])
            gate(xtq_dma[0][1], chunk_mm[20])
            gate(xtq_dma[1][0], chunk_mm[23])
            gate(xtq_dma[1][1], chunk_mm[23])
            gate(xtq_dma[2][0], chunk_mm[26])
            gate(xtq_dma[2][1], chunk_mm[26])
            gate(xtq_dma[3][0], chunk_mm[30])
            gate(xtq_dma[3][1], chunk_mm[30])

            # S PSUM -> SBUF fp16, split across DVE/ACT to unblock M fast
            for et in range(NO):
                if et < 6:
                    nc.vector.tensor_copy(Ssl[:, et, :], accs[et][:])
                else:
                    nc.scalar.activation(Ssl[:, et, :], accs[et][:], ACT_COPY)

            # ---- M_sl = W^T S_sl : lhsT = W[e_ch, a_tile], rhs = S_sl[e_ch, :]
            for at in range(NO):
                acc = psum.tile([P, 512], F32, tag="acc", name=f"macc_{at}")[:, :SL]
                for ch in range(NO):
                    mm = nc.tensor.matmul(
                        acc[:],
                        Wsb[:, ch, at * P : (at + 1) * P],
                        Ssl[:, ch, :],
                        start=(ch == 0),
                        stop=(ch == NO - 1),
                    )
                if at < 6:
                    nc.vector.tensor_copy(Msl[:, at, :], acc[:])
                else:
                    nc.scalar.activation(Msl[:, at, :], acc[:], ACT_COPY)

            # ---- outT_sl = M_sl^T X^T : stationary = M_sl half-columns,
            # moving = X^T 512-wide slices, quarter-major.
            for q in range(NQ):
                for h in range(2):
                    oaccs = [
                        psum.tile([P, 512], F32, tag="acc", name=f"oacc_{q}_{h}_{cg}")
                        for cg in range(2)
                    ]
                    for ch in range(NO):
                        for cg in range(2):
                            mm = nc.tensor.matmul(
                                oaccs[cg][:],
                                Msl[:, ch, h * P : (h + 1) * P],
                                Xtq[q][:, ch, cg * 512 : (cg + 1) * 512],
                                start=(ch == 0),
                                stop=(ch == NO - 1),
                            )

                    ot = stage.tile([P, 2, 512], F16, tag="ot")
                    nc.vector.tensor_copy(ot[:, 0, :], oaccs[0][:])
                    nc.scalar.activation(ot[:, 1, :], oaccs[1][:], ACT_COPY)
                    if q < 2:
                        # rings are still draining X^T here; use SWDGE
                        nc.gpsimd.dma_start(
                            o_out[h * P : (h + 1) * P, q * QW : (q + 1) * QW],
                            ot[:].rearrange("p c k -> p (c k)"),
                        )
                    else:
                        # per-cg writes on the (now idle) HWDGE rings so the
                        # final write starts as soon as its own copy lands
                        for cg in range(2):
                            weng = nc.sync if cg == 0 else nc.scalar
                            weng.dma_start(
                                o_out[
                                    h * P : (h + 1) * P,
                                    q * QW + cg * 512 : q * QW + (cg + 1) * 512,
                                ],
                                ot[:, cg, :],
                            )

    nc.finalize()
    return nc


def _get_compiled():
    global _compiled
    if _compiled is None:
        _compiled = _build()
    return _compiled


def kernel(hidden_states, queries, _trace=False, _trace_cores=None):
    x = np.ascontiguousarray(np.asarray(hidden_states, dtype=np.float32))
    w = np.ascontiguousarray(np.asarray(queries, dtype=np.float32))
    assert x.shape == (B, N, D) and w.shape == (D, D)

    nc = _get_compiled()
    w16 = w.astype(np.float16)
    x16 = [x[b].astype(np.float16) for b in range(B)]
    xt16 = [np.ascontiguousarray(x16[b].T) for b in range(B)]
    in_maps = []
    for c in range(NCORES):
        b, s = c // GROUP, c % GROUP
        in_maps.append(
            {
                "xf": np.ascontiguousarray(np.roll(x16[b], -s * SL, axis=1)),
                "xt": xt16[b],
                "wf": np.ascontiguousarray(np.roll(w16, -s * SL, axis=0)),
            }
        )

    res = run_bass_kernel_spmd(
        nc,
        in_maps,
        core_ids=list(range(NCORES)),
        trace=_trace,
        trace_cores=_trace_cores,
    )

    out = np.empty((B, N, D), dtype=np.float32)
    for c in range(NCORES):
        b, s = c // GROUP, c % GROUP
        out[b, :, s * SL : (s + 1) * SL] = res.results[c]["o_out"].T.astype(np.float32)

    if _trace:
        kernel.last_result = res
    return out


# revision 3
# speedup vs baseline: 1.0014x; 1.0014x over previous
"""Trainium2 Bass kernel for nn_DenseAttentionOneHead — pipelined variant.

out_b = X_b (W^T (X_b^T X_b)).  Column-sliced across 4 cores per batch
(256 output columns each), collective-free.  Per core:
  S_sl = X_b^T X_b[:, sl]          ([1024, 256])
  M_sl = W^T S_sl                  ([1024, 256])
  outT_sl = M_sl^T X_b^T           ([256, 4096], written fp16; host
                                    transposes back)

The kernel is HBM/PE co-limited (~20MB DMA, ~61us of matmul columns), so
everything hangs on keeping both engines continuously busy:
  - The X row stream owns the full HBM bandwidth during the S phase; the
    W and X^T loads are explicitly dependency-gated (add_dep_helper) on
    late S-phase matmuls so they drain in strict consumption order
    behind the stream instead of competing with it.
  - X^T arrives in quarter-major order and the out phase consumes it
    quarter by quarter, so out-phase compute starts ~6us after M.
  - The out phase computes outT with M_sl halves stationary and X^T as
    the 512-wide moving operand; output is fp16 (2MB not 4MB).
  - PE never idles >1us, so the HAM clock gate stays at 8/8.
"""

import numpy as np

import concourse.mybir as mybir
import concourse.tile as tile
from concourse import bacc
from concourse.bass import _add_dep_helper
from concourse.bass_utils import run_bass_kernel_spmd

F32 = mybir.dt.float32
F16 = mybir.dt.float16
P = 128
D = 1024
B = 2
N = 4096
NCORES = 8
GROUP = 4            # cores per batch
SL = D // GROUP      # 256-column slice per core
NO = D // P          # 8 blocks of 128 along D
NCH = N // P         # 32 row chunks of the batch
NQ = 4               # X^T quarters (1024 N-columns each)
QW = N // NQ         # 1024
ACT_COPY = mybir.ActivationFunctionType.Copy

_compiled = None


def _build():
    nc = bacc.Bacc(None, target_bir_lowering=False, debug=False, num_devices=NCORES)

    # xf arrives column-rotated per core (its 256 target columns first) and
    # wf row-rotated identically, so the same program computes every slice.
    xf = nc.dram_tensor("xf", [N, D], F16, kind="ExternalInput")
    xt = nc.dram_tensor("xt", [D, N], F16, kind="ExternalInput")
    wf = nc.dram_tensor("wf", [D, D], F16, kind="ExternalInput")
    o_out = nc.dram_tensor("o_out", [SL, N], F16, kind="ExternalOutput")

    xfv = xf.rearrange("(c p) d -> p c d", p=P)      # [128, 32, 1024]
    xtv = xt.rearrange("(c p) n -> p c n", p=P)      # [128, 8, 4096]
    wfv = wf.rearrange("(c p) a -> p c a", p=P)      # [128, 8, 1024]

    # X row-stream pieces: 1-chunk pieces through the DMA ramp-up, then
    # 2-chunk (512KB) pieces.  Uniform small pieces keep the two rings'
    # delivery aligned with consumption order (a large piece at the head
    # of one ring stalls the in-order consumer while the other ring
    # races ahead).
    piece_chunks = [1] * 8 + [2] * 12
    assert sum(piece_chunks) == NCH

    with tile.TileContext(nc) as tc:
        with (
            tc.tile_pool(name="big", bufs=1) as big,
            tc.tile_pool(name="xin", bufs=12) as xin,
            tc.tile_pool(name="stage", bufs=4) as stage,
            tc.tile_pool(name="psum", bufs=8, space="PSUM") as psum,
        ):
            Wsb = big.tile([P, NO, D], F16, tag="W")        # W [e, a], 2MB
            Ssl = big.tile([P, NO, SL], F16, tag="Ssl")     # S_sl [e, d_sl]
            Msl = big.tile([P, NO, SL], F16, tag="Msl")     # M_sl [a, d_sl]
            Xtq = [
                big.tile([P, NO, QW], F16, tag=f"Xtq{q}", name=f"Xtq{q}")
                for q in range(NQ)
            ]                                               # X^T quarters, 4x2MB

            # PE warm-up: ~3.5us of throwaway matmuls with no DMA dependency
            # run during the framework preamble / first-piece latency, so the
            # HAM clock gate reaches 8/8 before the first real matmul.
            dummy = big.tile([P, P], F16, tag="dummy")
            nc.gpsimd.memset(dummy[:], 0.0)
            dacc = psum.tile([P, 512], F32, tag="acc", name="warmup")
            for i in range(16):
                nc.tensor.matmul(
                    dacc[:, :P], dummy[:], dummy[:], start=(i == 0), stop=(i == 15)
                )

            # ---- S_sl = X^T X[:, sl]: stream X row-chunks on both rings
            accs = [
                psum.tile([P, 512], F32, tag="acc", name=f"sacc_{et}")[:, :SL]
                for et in range(NO)
            ]
            chunk_mm = {}       # chunk index -> its last matmul instruction
            ch = 0
            for pi, nch in enumerate(piece_chunks):
                xp = xin.tile([P, 2, D], F16, tag="xp")
                if pi == 0:
                    # Split the first chunk into column halves so its first
                    # matmuls start ~1.7us earlier (smaller first transfer).
                    nc.sync.dma_start(xp[:, 0, : D // 2], xfv[:, 0, : D // 2])
                    nc.scalar.dma_start(xp[:, 0, D // 2 :], xfv[:, 0, D // 2 :])
                else:
                    eng = nc.sync if pi % 2 == 0 else nc.scalar
                    eng.dma_start(xp[:, :nch, :], xfv[:, ch : ch + nch, :])
                for c in range(nch):
                    for et in range(NO):
                        mm = nc.tensor.matmul(
                            accs[et][:],
                            xp[:, c, et * P : (et + 1) * P],
                            xp[:, c, :SL],
                            start=(ch == 0),
                            stop=(ch == NCH - 1),
                        )
                    chunk_mm[ch] = mm
                    ch += 1

            # W and X^T loads, gated behind the stream in consumption order.
            gate = lambda dma, mm: _add_dep_helper(
                dma.ins, mm.ins, sync=True, reason="serialize HBM behind X stream"
            )
            # W and X^T loads trail the stream in strict consumption order
            # (each quarter split across both rings so quarters complete
            # sequentially).  q0/q1 go eagerly right behind W; q2/q3 are
            # gated off M-phase progress, moving ~4MB out of the window
            # where both HBM-pair cores burst simultaneously.
            d = nc.sync.dma_start(Wsb[:, :, : D // 2], wfv[:, :, : D // 2])
            gate(d, chunk_mm[16])
            d = nc.scalar.dma_start(Wsb[:, :, D // 2 :], wfv[:, :, D // 2 :])
            gate(d, chunk_mm[18])
            xtq_dma = []
            for q in range(NQ):
                da = nc.sync.dma_start(
                    Xtq[q][:, : NO // 2, :],
                    xtv[:, : NO // 2, q * QW : (q + 1) * QW],
                )
                db = nc.scalar.dma_start(
                    Xtq[q][:, NO // 2 :, :],
                    xtv[:, NO // 2 :, q * QW : (q + 1) * QW],
                )
                xtq_dma.append((da, db))
            gate(xtq_dma[0][0], chunk_mm[20])
            gate(xtq_dma[0][1], chunk_mm[20])
            gate(xtq_dma[1][0], chunk_mm[23])
            gate(xtq_dma[1][1], chunk_mm[23])
            gate(xtq_dma[2][0], chunk_mm[26])
            gate(xtq_dma[2][1], chunk_mm[26])
            gate(xtq_dma[3][0], chunk_mm[30])
            gate(xtq_dma[3][1], chunk_mm[30])

            # S PSUM -> SBUF fp16, split across DVE/ACT to unblock M fast
            for et in range(NO):
                if et < 6:
                    nc.vector.tensor_copy(Ssl[:, et, :], accs[et][:])
                else:
                    nc.scalar.activation(Ssl[:, et, :], accs[et][:], ACT_COPY)

            # ---- M_sl = W^T S_sl : lhsT = W[e_ch, a_tile], rhs = S_sl[e_ch, :]
            for at in range(NO):
                acc = psum.tile([P, 512], F32, tag="acc", name=f"macc_{at}")[:, :SL]
                for ch in range(NO):
                    mm = nc.tensor.matmul(
                        acc[:],
                        Wsb[:, ch, at * P : (at + 1) * P],
                        Ssl[:, ch, :],
                        start=(ch == 0),
                        stop=(ch == NO - 1),
                    )
                if at < 6:
                    nc.vector.tensor_copy(Msl[:, at, :], acc[:])
                else:
                    nc.scalar.activation(Msl[:, at, :], acc[:], ACT_COPY)

            # ---- outT_sl = M_sl^T X^T : stationary = M_sl half-columns,
            # moving = X^T 512-wide slices, quarter-major.
            for q in range(NQ):
                for h in range(2):
                    oaccs = [
                        psum.tile([P, 512], F32, tag="acc", name=f"oacc_{q}_{h}_{cg}")
                        for cg in range(2)
                    ]
                    for ch in range(NO):
                        for cg in range(2):
                            mm = nc.tensor.matmul(
                                oaccs[cg][:],
                                Msl[:, ch, h * P : (h + 1) * P],
                                Xtq[q][:, ch, cg * 512 : (cg + 1) * 512],
                                start=(ch == 0),
                                stop=(ch == NO - 1),
                            )

                    ot = stage.tile([P, 2, 512], F16, tag="ot")
                    nc.vector.tensor_copy(ot[:, 0, :], oaccs[0][:])
                    nc.scalar.activation(ot[:, 1, :], oaccs[1][:], ACT_COPY)
                    if q < 2:
                        # rings are still draining X^T here; use SWDGE
                        nc.gpsimd.dma_start(
                            o_out[h * P : (h + 1) * P, q * QW : (q + 1) * QW],
                            ot[:].rearrange("p c k -> p (c k)"),
                        )
                    else:
                        # per-cg writes on the (now idle) HWDGE rings so the
                        # final write starts as soon as its own copy lands
                        for cg in range(2):
                            weng = nc.sync if cg == 0 else nc.scalar
                            weng.dma_start(
                                o_out[
                                    h * P : (h + 1) * P,
                                    q * QW + cg * 512 : q * QW + (cg + 1) * 512,
                                ],
                                ot[:, cg, :],
                            )

    nc.finalize()
    return nc


def _get_compiled():
    global _compiled
    if _compiled is None:
        _compiled = _build()
    return _compiled


def kernel(hidden_states, queries, _trace=False, _trace_cores=None):
    x = np.ascontiguousarray(np.asarray(hidden_states, dtype=np.float32))
    w = np.ascontiguousarray(np.asarray(queries, dtype=np.float32))
    assert x.shape == (B, N, D) and w.shape == (D, D)

    nc = _get_compiled()
    w16 = w.astype(np.float16)
    x16 = [x[b].astype(np.float16) for b in range(B)]
    xt16 = [np.ascontiguousarray(x16[b].T) for b in range(B)]
    in_maps = []
    for c in range(NCORES):
        b, s = c // GROUP, c % GROUP
        in_maps.append(
            {
                "xf": np.ascontiguousarray(np.roll(x16[b], -s * SL, axis=1)),
                "xt": xt16[b],
                "wf": np.ascontiguousarray(np.roll(w16, -s * SL, axis=0)),
            }
        )

    res = run_bass_kernel_spmd(
        nc,
        in_maps,
        core_ids=list(range(NCORES)),
        trace=_trace,
        trace_cores=_trace_cores,
    )

    out = np.empty((B, N, D), dtype=np.float32)
    for c in range(NCORES):
        b, s = c // GROUP, c % GROUP
        out[b, :, s * SL : (s + 1) * SL] = res.results[c]["o_out"].T.astype(np.float32)

    if _trace:
        kernel.last_result = res
    return out
